# revision 9
# baseline (speedup 1.0000x reference)
# BitLinear (eval path) Trainium2 kernel: ternary weight quant + int8 activation
# quant + dense matmul, tensor-parallel over 8 NeuronCores.
#
# Math (per reference):
#   w_scale[o] = max(mean_k |W[o,k]|, EPS)
#   w_quant    = clip(round(W / w_scale), -1, 1)            (ternary)
#   x_scale[t] = max(max_k |x[t,k]| / 127, EPS)
#   x_quant    = round(x / x_scale)                          (int8 range)
#   out[t,o]   = (sum_k x_quant[t,k] * w_quant[o,k]) * x_scale[t] * w_scale[o] + bias[o]
#
# The integer sum is exact on the PE: w_quant and x_quant (|v| <= 127) are exact
# in bf16, partials are exact in the fp32 PSUM accumulator (|sum| <= 127*4096 < 2^24).
#
# Sharding: 2 token groups x 4 out-feature groups = 8 cores.  Both x and W are
# passed in natural row-major layout ([t,k] / [o,k]); quantization runs with the
# reduction dim on the free axis so scales are per-partition (cheap TS/ACT ops),
# then the quantized bf16 tiles are transposed to K-major with the DMA xbar
# transpose engine for the matmul.  Bias is added on the host (exact final fp32
# add; it is the last op of the reference).
#
# Scheduling: PE-bound main loop (2048 back-to-back N=512 bf16 matmuls,
# ~216ns each).  The W phase is split: as soon as weight tiles 0..3 are
# quantized+transposed, chunks 0..3 run their oc=0 matmul groups while weight
# tiles 4..7 quantize underneath; their oc=1 groups follow.  Per main-loop
# iteration the issue order keeps every queue free of head-of-line blocking:
# [matmuls, STT, out-DMA], in-DMA(ch+5), quant-chain(ch+4) (transpose last).
import numpy as np

import concourse.bacc as bacc
import concourse.bass as bass
import concourse.tile as tile
from concourse import mybir
from concourse.bass_utils import run_bass_kernel_spmd
from concourse.masks import make_identity

F32 = mybir.dt.float32
BF16 = mybir.dt.bfloat16

EPS = 1e-5
MAGIC = 12582912.0  # 1.5 * 2^23: (x + MAGIC) - MAGIC == rint(x) for |x| < 2^22

# Full-problem shapes (hardcoded per contract).
B, S, I, O = 4, 2048, 4096, 4096
T_FULL = B * S  # 8192 tokens
TSPLIT, OSPLIT = 2, 4  # token groups x out-feature groups = 8 cores
N_CORES = TSPLIT * OSPLIT

A = mybir.AluOpType
AF = mybir.ActivationFunctionType


def build_nc(K=I, TO=O // OSPLIT, TT=T_FULL // TSPLIT):
    """Per-core program: x [TT, K] f32, w [TO, K] f32 -> out [TT, TO] f32."""
    KT = K // 128  # 32 k subtiles
    NOT = TO // 128  # 8 weight-row tiles
    NCH = TT // 128  # 32 token chunks
    NOC = TO // 512  # 2 psum column chunks per token chunk
    EARLY = 4  # chunks whose oc groups are split around the W phase tail

    nc = bacc.Bacc("TRN2", target_bir_lowering=False, debug=False)
    x_d = nc.dram_tensor("x", [TT, K], F32, kind="ExternalInput").ap()
    w_d = nc.dram_tensor("w", [TO, K], F32, kind="ExternalInput").ap()
    out_d = nc.dram_tensor("out", [TT, TO], F32, kind="ExternalOutput").ap()

    with tile.TileContext(nc) as tc:
        with (
            tc.tile_pool(name="blk", bufs=4) as p_blk,  # f32 staging (x chunks / w tiles)
            tc.tile_pool(name="yb", bufs=2) as p_yb,  # bf16 quantized, pre-transpose
            tc.tile_pool(name="absw", bufs=1) as p_absw,
            tc.tile_pool(name="wq", bufs=1) as p_wq,
            tc.tile_pool(name="xq", bufs=4) as p_xq,  # K-major xq chunks
            tc.tile_pool(name="osb", bufs=4) as p_osb,
            tc.tile_pool(name="xs", bufs=7) as p_xs,
            tc.tile_pool(name="small", bufs=4) as p_small,
            tc.tile_pool(name="const", bufs=1) as p_const,
            tc.tile_pool(name="ps_mm", bufs=6, space="PSUM") as ps_mm,
            tc.tile_pool(name="ps_warm", bufs=1, space="PSUM") as ps_warm_pool,
            tc.tile_pool(name="ps_misc", bufs=1, space="PSUM") as ps_misc,
        ):
            ident = p_const.tile([128, 128], F32)
            make_identity(nc, ident[:])
            junk = p_const.tile([128, 128], BF16)
            nc.vector.memset(junk[:], 1.0)
            # Resident quantized weights, K-major: [k%128, ot, kt, o%128].
            wq = p_wq.tile([128, NOT, KT, 128], BF16)
            ws_bc = p_const.tile([128, TO], F32)  # row 0 filled, then broadcast

            # ---------------- W phase ----------------
            def w_load(ot):
                w_f32 = p_blk.tile([128, K], F32, tag="blk")
                nc.sync.dma_start(
                    out=w_f32[:], in_=w_d[ot * 128 : (ot + 1) * 128, :]
                )
                return w_f32

            def w_quant(ot, w_f32):
                absw = p_absw.tile([128, K], F32, tag="absw")
                ws_sum = p_small.tile([128, 1], F32, tag="wsum")
                nc.scalar.activation(
                    out=absw[:], in_=w_f32[:], func=AF.Abs, accum_out=ws_sum[:]
                )
                ws_col = p_small.tile([128, 1], F32, tag="wscol")
                nc.vector.tensor_scalar(
                    out=ws_col[:], in0=ws_sum[:], scalar1=1.0 / K, scalar2=EPS,
                    op0=A.mult, op1=A.max,
                )
                rws_col = p_small.tile([128, 1], F32, tag="rwscol")
                nc.vector.reciprocal(rws_col[:], ws_col[:])
                # y = rint(w * rws) + MAGIC  (fp32, in place)
                nc.vector.tensor_scalar(
                    out=w_f32[:], in0=w_f32[:], scalar1=rws_col[:], scalar2=MAGIC,
                    op0=A.mult, op1=A.add,
                )
                yb = p_yb.tile([128, K], BF16, tag="yb")
                if ot % 2 == 0:
                    # ACT path: yb = y - MAGIC -> bf16 (exact affine), clip on DVE
                    nc.scalar.activation(
                        out=yb[:], in_=w_f32[:], func=AF.Copy, bias=-MAGIC
                    )
                    nc.vector.tensor_scalar(
                        out=yb[:], in0=yb[:], scalar1=1.0, scalar2=-1.0,
                        op0=A.min, op1=A.max,
                    )
                else:
                    # DVE path: two fp32 tensor_scalar passes
                    nc.vector.tensor_scalar(
                        out=w_f32[:], in0=w_f32[:], scalar1=MAGIC, scalar2=1.0,
                        op0=A.subtract, op1=A.min,
                    )
                    nc.vector.tensor_scalar(
                        out=yb[:], in0=w_f32[:], scalar1=-1.0, scalar2=None,
                        op0=A.max,
                    )
                # transpose [o, k] -> [k%128, kt, o] into the resident block
                nc.sync.dma_start_transpose(wq[:, ot, :, :], yb[:])
                # ws column -> ws_bc row-0 segment (PE transpose via identity)
                ps_r = ps_misc.tile([1, 128], F32, tag="wsrow")
                nc.tensor.transpose(ps_r[:], ws_col[:], ident[:])
                nc.vector.tensor_copy(
                    ws_bc[0:1, ot * 128 : (ot + 1) * 128], ps_r[:]
                )

            # ---------- x-chunk load + quantize ----------
            def x_load(ch):
                x_f32 = p_blk.tile([128, K], F32, tag="blk")
                nc.sync.dma_start(out=x_f32[:], in_=x_d[ch * 128 : (ch + 1) * 128, :])
                return x_f32

            def x_quant(ch, x_f32):
                am = p_small.tile([128, 1], F32, tag="am")
                nc.vector.tensor_reduce(
                    out=am[:], in_=x_f32[:], axis=mybir.AxisListType.X,
                    op=A.max, apply_absolute_value=True,
                )
                xs_col = p_xs.tile([128, 1], F32, tag="xs")
                nc.vector.tensor_scalar(
                    out=xs_col[:], in0=am[:], scalar1=1.0 / 127.0, scalar2=EPS,
                    op0=A.mult, op1=A.max,
                )
                rxs_col = p_small.tile([128, 1], F32, tag="rxs")
                nc.vector.reciprocal(rxs_col[:], xs_col[:])
                # y = rint(x * rxs) + MAGIC  (fp32, in place)
                nc.vector.tensor_scalar(
                    out=x_f32[:], in0=x_f32[:], scalar1=rxs_col[:], scalar2=MAGIC,
                    op0=A.mult, op1=A.add,
                )
                # yb = y - MAGIC -> bf16 (exact: integers, |v| <= 127)
                yb = p_yb.tile([128, K], BF16, tag="yb")
                nc.scalar.activation(out=yb[:], in_=x_f32[:], func=AF.Copy, bias=-MAGIC)
                # transpose [t, k] -> [k%128, kt, t]
                xq_kt = p_xq.tile([128, KT, 128], BF16, tag="xq")
                nc.sync.dma_start_transpose(xq_kt[:], yb[:])
                return xq_kt, xs_col

            # ---------- matmul group (one psum bank) + epilogue ----------
            def mm_group(ch, oc, xq_kt, xs_col):
                pm = ps_mm.tile([128, 512], F32, tag="mm")
                for kt in range(KT):
                    nc.tensor.matmul(
                        pm[:],
                        xq_kt[:, kt, :],
                        wq[:, oc * 4 : (oc + 1) * 4, kt, :],
                        start=(kt == 0),
                        stop=(kt == KT - 1),
                    )
                osb = p_osb.tile([128, 512], F32, tag="osb")
                # out = (psum * xs[t]) * ws[o]
                nc.vector.scalar_tensor_tensor(
                    out=osb[:],
                    in0=pm[:],
                    scalar=xs_col[:],
                    in1=ws_bc[:, oc * 512 : (oc + 1) * 512],
                    op0=A.mult,
                    op1=A.mult,
                )
                nc.sync.dma_start(
                    out=out_d[
                        ch * 128 : (ch + 1) * 128, oc * 512 : (oc + 1) * 512
                    ],
                    in_=osb[:],
                )

            # ---------------- schedule ----------------
            # Head DMAs interleaved so ring aliasing never crosses the early
            # matmul groups: w0 w1 x0 x1 w2 w3 x2 x3 w4..w7 on the 4-deep blk
            # ring makes every DMA wait only on a quant chain issued before it.
            w_tiles = {}
            x_tiles = {}
            w_tiles[0], w_tiles[1] = w_load(0), w_load(1)
            x_tiles[0], x_tiles[1] = x_load(0), x_load(1)
            w_tiles[2], w_tiles[3] = w_load(2), w_load(3)
            x_tiles[2], x_tiles[3] = x_load(2), x_load(3)
            for ot in range(4, NOT):
                w_tiles[ot] = w_load(ot)
            # Quantize weight tiles 0..3 (left oc half), broadcast its scales.
            for ot in range(4):
                w_quant(ot, w_tiles.pop(ot))
            nc.gpsimd.partition_broadcast(ws_bc[:, 0:512], ws_bc[0:1, 0:512])
            quants = {}
            for ch in range(4):
                quants[ch] = x_quant(ch, x_tiles.pop(ch))
            # PE warmup: junk matmuls gated on an early wq transpose bridge the
            # HAM cold-ramp right before the first real groups.
            ps_warm = ps_warm_pool.tile([128, 128], F32, tag="warm")
            for r in range(24):
                nc.tensor.matmul(
                    ps_warm[:], junk[:], wq[:, 2, r % KT, :], start=True, stop=True
                )
            warm_sink = p_small.tile([128, 1], F32, tag="wsink")
            nc.vector.tensor_copy(warm_sink[:], ps_warm[:, 0:1])
            # Early oc=0 groups run while weight tiles 4..7 quantize.
            for ch in range(EARLY):
                mm_group(ch, 0, *quants[ch])
            for ot in range(4, NOT):
                w_quant(ot, w_tiles.pop(ot))
            nc.gpsimd.partition_broadcast(ws_bc[:, 512:1024], ws_bc[0:1, 512:1024])
            for ch in (4, 5):
                x_tiles[ch] = x_load(ch)
            quants[4] = x_quant(4, x_tiles.pop(4))
            for ch in (6, 7):
                x_tiles[ch] = x_load(ch)
            quants[5] = x_quant(5, x_tiles.pop(5))
            # Early oc=1 groups (free the early xq tiles in order).
            for ch in range(EARLY):
                mm_group(ch, 1, *quants.pop(ch))
            x_tiles[8] = x_load(8)
            for ch in (6, 7):
                quants[ch] = x_quant(ch, x_tiles.pop(ch))
            # Steady state.
            for ch in range(EARLY, NCH):
                xq_kt, xs_col = quants.pop(ch)
                mm_group(ch, 0, xq_kt, xs_col)
                mm_group(ch, 1, xq_kt, xs_col)
                if ch + 5 < NCH:
                    x_tiles[ch + 5] = x_load(ch + 5)
                if ch + 4 < NCH:
                    quants[ch + 4] = x_quant(ch + 4, x_tiles.pop(ch + 4))
    nc.compile()
    return nc


_NC_CACHE = {}
TRACE = False
LAST_EXEC_NS = None


def _get_nc():
    key = "full"
    if key not in _NC_CACHE:
        _NC_CACHE[key] = build_nc()
    return _NC_CACHE[key]


def _run(x, weight, bias, trace=False):
    global LAST_EXEC_NS
    x = np.asarray(x, dtype=np.float32)
    weight = np.asarray(weight, dtype=np.float32)
    bias = np.asarray(bias, dtype=np.float32)

    x2 = x.reshape(T_FULL, I)
    TT = T_FULL // TSPLIT
    TO = O // OSPLIT
    in_maps = []
    for c in range(N_CORES):
        ti, oj = divmod(c, OSPLIT)
        in_maps.append(
            {
                "x": np.ascontiguousarray(x2[ti * TT : (ti + 1) * TT]),
                "w": np.ascontiguousarray(weight[oj * TO : (oj + 1) * TO]),
            }
        )

    nc = _get_nc()
    res = run_bass_kernel_spmd(
        nc, in_maps, core_ids=list(range(N_CORES)), trace=trace
    )
    LAST_EXEC_NS = res.exec_time_ns

    out = np.empty((T_FULL, O), dtype=np.float32)
    for c in range(N_CORES):
        ti, oj = divmod(c, OSPLIT)
        out[ti * TT : (ti + 1) * TT, oj * TO : (oj + 1) * TO] = res.results[c]["out"]
    out += bias  # exact final fp32 add (reference's last op)
    return out.reshape(B, S, O)


def kernel(x, weight, bias):
    return _run(x, weight, bias, trace=False)


def kernel_traced(x, weight, bias):
    _run(x, weight, bias, trace=True)
    return LAST_EXEC_NS


# revision 12
# speedup vs baseline: 1.0862x; 1.0862x over previous
# BitLinear (eval path) Trainium2 kernel: ternary weight quant + int8 activation
# quant + dense matmul, tensor-parallel over 8 NeuronCores.
#
# Math (per reference):
#   w_scale[o] = max(mean_k |W[o,k]|, EPS)
#   w_quant    = clip(round(W / w_scale), -1, 1)            (ternary)
#   x_scale[t] = max(max_k |x[t,k]| / 127, EPS)
#   x_quant    = round(x / x_scale)                          (int8 range)
#   out[t,o]   = (sum_k x_quant[t,k] * w_quant[o,k]) * x_scale[t] * w_scale[o] + bias[o]
#
# The integer sum is exact on the PE: w_quant and x_quant (|v| <= 127) are exact
# in bf16, partials are exact in the fp32 PSUM accumulator (|sum| <= 127*4096 < 2^24).
#
# Sharding: 2 token groups x 4 out-feature groups = 8 cores.  Both x and W are
# passed in natural row-major layout ([t,k] / [o,k]); quantization runs with the
# reduction dim on the free axis so scales are per-partition (cheap TS/ACT ops),
# then the quantized bf16 tiles are transposed to K-major with the DMA xbar
# transpose engine for the matmul.  Bias is added on the host (exact final fp32
# add; it is the last op of the reference).
#
# Scheduling: PE-bound main loop (2048 back-to-back N=512 bf16 matmuls,
# ~216ns each).  Per main-loop iteration the issue order keeps every queue
# free of head-of-line blocking: [matmuls(ch), STT(ch), out(ch)] then
# in-DMA(ch+6) then quant-chain(ch+4) (transpose dispatched last on sync).
# All input DMAs stay on the sync queue (DMA-in dispatched from the scalar
# queue alongside ACT compute proved flaky on hardware).
import numpy as np

import concourse.bacc as bacc
import concourse.bass as bass
import concourse.tile as tile
from concourse import mybir
from concourse.bass_utils import run_bass_kernel_spmd
from concourse.masks import make_identity

F32 = mybir.dt.float32
BF16 = mybir.dt.bfloat16

EPS = 1e-5
MAGIC = 12582912.0  # 1.5 * 2^23: (x + MAGIC) - MAGIC == rint(x) for |x| < 2^22

# Full-problem shapes (hardcoded per contract).
B, S, I, O = 4, 2048, 4096, 4096
T_FULL = B * S  # 8192 tokens
TSPLIT, OSPLIT = 2, 4  # token groups x out-feature groups = 8 cores
N_CORES = TSPLIT * OSPLIT

A = mybir.AluOpType
AF = mybir.ActivationFunctionType


def build_nc(K=I, TO=O // OSPLIT, TT=T_FULL // TSPLIT):
    """Per-core program: x [TT, K] f32, w [TO, K] f32 -> out [TT, TO] f32."""
    KT = K // 128  # 32 k subtiles
    NOT = TO // 128  # 8 weight-row tiles
    NCH = TT // 128  # 32 token chunks
    NOC = TO // 512  # 2 psum column chunks per token chunk
    QPRE = 4  # quantization lookahead (chunks)
    LPRE = 5  # x-load lookahead (chunks)

    nc = bacc.Bacc("TRN2", target_bir_lowering=False, debug=False)
    x_d = nc.dram_tensor("x", [TT, K], F32, kind="ExternalInput").ap()
    w_d = nc.dram_tensor("w", [TO, K], F32, kind="ExternalInput").ap()
    out_d = nc.dram_tensor("out", [TT, TO], F32, kind="ExternalOutput").ap()

    with tile.TileContext(nc) as tc:
        with (
            tc.tile_pool(name="blk", bufs=4) as p_blk,  # f32 staging (x chunks / w tiles)
            tc.tile_pool(name="yb", bufs=2) as p_yb,  # bf16 quantized, pre-transpose
            tc.tile_pool(name="absw", bufs=1) as p_absw,
            tc.tile_pool(name="wq", bufs=1) as p_wq,
            tc.tile_pool(name="xq", bufs=3) as p_xq,  # K-major xq chunks
            tc.tile_pool(name="osb", bufs=2) as p_osb,
            tc.tile_pool(name="xs", bufs=QPRE + 2) as p_xs,
            tc.tile_pool(name="small", bufs=4) as p_small,
            tc.tile_pool(name="const", bufs=1) as p_const,
            tc.tile_pool(name="ps_mm", bufs=6, space="PSUM") as ps_mm,
            tc.tile_pool(name="ps_warm", bufs=1, space="PSUM") as ps_warm_pool,
            tc.tile_pool(name="ps_misc", bufs=1, space="PSUM") as ps_misc,
        ):
            ident = p_const.tile([128, 128], F32)
            make_identity(nc, ident[:])
            junk = p_const.tile([128, 128], BF16)
            nc.vector.memset(junk[:], 1.0)
            # Resident quantized weights, K-major: [k%128, ot, kt, o%128].
            wq = p_wq.tile([128, NOT, KT, 128], BF16)
            ws_bc = p_const.tile([128, TO], F32)  # row 0 filled, then broadcast

            # ---------------- W phase ----------------
            def w_load(ot):
                w_f32 = p_blk.tile([128, K], F32, tag="blk")
                nc.sync.dma_start(
                    out=w_f32[:], in_=w_d[ot * 128 : (ot + 1) * 128, :]
                )
                return w_f32

            def w_quant(ot, w_f32):
                absw = p_absw.tile([128, K], F32, tag="absw")
                ws_sum = p_small.tile([128, 1], F32, tag="wsum")
                nc.scalar.activation(
                    out=absw[:], in_=w_f32[:], func=AF.Abs, accum_out=ws_sum[:]
                )
                ws_col = p_small.tile([128, 1], F32, tag="wscol")
                nc.vector.tensor_scalar(
                    out=ws_col[:], in0=ws_sum[:], scalar1=1.0 / K, scalar2=EPS,
                    op0=A.mult, op1=A.max,
                )
                rws_col = p_small.tile([128, 1], F32, tag="rwscol")
                nc.vector.reciprocal(rws_col[:], ws_col[:])
                # y = rint(w * rws) + MAGIC  (fp32, in place)
                nc.vector.tensor_scalar(
                    out=w_f32[:], in0=w_f32[:], scalar1=rws_col[:], scalar2=MAGIC,
                    op0=A.mult, op1=A.add,
                )
                yb = p_yb.tile([128, K], BF16, tag="yb")
                if ot % 2 == 0:
                    # ACT path: yb = y - MAGIC -> bf16 (exact affine), clip on DVE
                    nc.scalar.activation(
                        out=yb[:], in_=w_f32[:], func=AF.Copy, bias=-MAGIC
                    )
                    nc.vector.tensor_scalar(
                        out=yb[:], in0=yb[:], scalar1=1.0, scalar2=-1.0,
                        op0=A.min, op1=A.max,
                    )
                else:
                    # DVE path: two fp32 tensor_scalar passes
                    nc.vector.tensor_scalar(
                        out=w_f32[:], in0=w_f32[:], scalar1=MAGIC, scalar2=1.0,
                        op0=A.subtract, op1=A.min,
                    )
                    nc.vector.tensor_scalar(
                        out=yb[:], in0=w_f32[:], scalar1=-1.0, scalar2=None,
                        op0=A.max,
                    )
                # transpose [o, k] -> [k%128, kt, o] into the resident block
                nc.sync.dma_start_transpose(wq[:, ot, :, :], yb[:])
                # ws column -> ws_bc row-0 segment (PE transpose via identity)
                ps_r = ps_misc.tile([1, 128], F32, tag="wsrow")
                nc.tensor.transpose(ps_r[:], ws_col[:], ident[:])
                nc.vector.tensor_copy(
                    ws_bc[0:1, ot * 128 : (ot + 1) * 128], ps_r[:]
                )

            # ---------- x-chunk load + quantize ----------
            def x_load(ch):
                x_f32 = p_blk.tile([128, K], F32, tag="blk")
                nc.sync.dma_start(out=x_f32[:], in_=x_d[ch * 128 : (ch + 1) * 128, :])
                return x_f32

            def x_quant(ch, x_f32):
                am = p_small.tile([128, 1], F32, tag="am")
                nc.vector.tensor_reduce(
                    out=am[:], in_=x_f32[:], axis=mybir.AxisListType.X,
                    op=A.max, apply_absolute_value=True,
                )
                xs_col = p_xs.tile([128, 1], F32, tag="xs")
                nc.vector.tensor_scalar(
                    out=xs_col[:], in0=am[:], scalar1=1.0 / 127.0, scalar2=EPS,
                    op0=A.mult, op1=A.max,
                )
                rxs_col = p_small.tile([128, 1], F32, tag="rxs")
                nc.vector.reciprocal(rxs_col[:], xs_col[:])
                # y = rint(x * rxs) + MAGIC  (fp32, in place)
                nc.vector.tensor_scalar(
                    out=x_f32[:], in0=x_f32[:], scalar1=rxs_col[:], scalar2=MAGIC,
                    op0=A.mult, op1=A.add,
                )
                # yb = y - MAGIC -> bf16 (exact: integers, |v| <= 127)
                yb = p_yb.tile([128, K], BF16, tag="yb")
                nc.scalar.activation(out=yb[:], in_=x_f32[:], func=AF.Copy, bias=-MAGIC)
                # transpose [t, k] -> [k%128, kt, t]
                xq_kt = p_xq.tile([128, KT, 128], BF16, tag="xq")
                nc.sync.dma_start_transpose(xq_kt[:], yb[:])
                return xq_kt, xs_col

            # ---------- main chunk: matmuls + epilogue ----------
            def x_mainloop(ch, xq_kt, xs_col):
                osb = p_osb.tile([128, TO], F32, tag="osb")
                for oc in range(NOC):
                    pm = ps_mm.tile([128, 512], F32, tag="mm")
                    for kt in range(KT):
                        nc.tensor.matmul(
                            pm[:],
                            xq_kt[:, kt, :],
                            wq[:, oc * 4 : (oc + 1) * 4, kt, :],
                            start=(kt == 0),
                            stop=(kt == KT - 1),
                        )
                    # out = (psum * xs[t]) * ws[o]
                    nc.vector.scalar_tensor_tensor(
                        out=osb[:, oc * 512 : (oc + 1) * 512],
                        in0=pm[:],
                        scalar=xs_col[:],
                        in1=ws_bc[:, oc * 512 : (oc + 1) * 512],
                        op0=A.mult,
                        op1=A.mult,
                    )
                nc.scalar.dma_start(
                    out=out_d[ch * 128 : (ch + 1) * 128, :], in_=osb[:]
                )

            # ---------------- schedule ----------------
            # All W input DMAs first (nothing blocks them in the sync queue).
            w_tiles = [w_load(ot) for ot in range(NOT)]
            x_tiles = {ch: x_load(ch) for ch in range(2)}
            for ot in range(NOT):
                w_quant(ot, w_tiles[ot])
            nc.gpsimd.partition_broadcast(ws_bc[:], ws_bc[0:1, :])
            quants = [x_quant(0, x_tiles.pop(0)), x_quant(1, x_tiles.pop(1))]
            for ch in (2, 3):
                x_tiles[ch] = x_load(ch)
            for ch in (2, 3):
                quants.append(x_quant(ch, x_tiles.pop(ch)))
            for ch in (4, 5):
                x_tiles[ch] = x_load(ch)
            # PE warmup: junk matmuls gated on the second-to-last wq transpose,
            # bridging the HAM cold-ramp right before the main stream starts.
            ps_warm = ps_warm_pool.tile([128, 128], F32, tag="warm")
            for r in range(24):
                nc.tensor.matmul(
                    ps_warm[:], junk[:], wq[:, NOT - 2, r % KT, :],
                    start=True, stop=True,
                )
            warm_sink = p_small.tile([128, 1], F32, tag="wsink")
            nc.vector.tensor_copy(warm_sink[:], ps_warm[:, 0:1])
            for ch in range(NCH):
                xq_kt, xs_col = quants.pop(0)
                x_mainloop(ch, xq_kt, xs_col)
                if ch + LPRE + 1 < NCH:
                    x_tiles[ch + LPRE + 1] = x_load(ch + LPRE + 1)
                if ch + QPRE < NCH:
                    quants.append(x_quant(ch + QPRE, x_tiles.pop(ch + QPRE)))
    nc.compile()
    return nc


_NC_CACHE = {}
TRACE = False
LAST_EXEC_NS = None


def _get_nc():
    key = "full"
    if key not in _NC_CACHE:
        _NC_CACHE[key] = build_nc()
    return _NC_CACHE[key]


def _run(x, weight, bias, trace=False):
    global LAST_EXEC_NS
    x = np.asarray(x, dtype=np.float32)
    weight = np.asarray(weight, dtype=np.float32)
    bias = np.asarray(bias, dtype=np.float32)

    x2 = x.reshape(T_FULL, I)
    TT = T_FULL // TSPLIT
    TO = O // OSPLIT
    in_maps = []
    for c in range(N_CORES):
        ti, oj = divmod(c, OSPLIT)
        in_maps.append(
            {
                "x": np.ascontiguousarray(x2[ti * TT : (ti + 1) * TT]),
                "w": np.ascontiguousarray(weight[oj * TO : (oj + 1) * TO]),
            }
        )

    nc = _get_nc()
    res = run_bass_kernel_spmd(
        nc, in_maps, core_ids=list(range(N_CORES)), trace=trace
    )
    LAST_EXEC_NS = res.exec_time_ns

    out = np.empty((T_FULL, O), dtype=np.float32)
    for c in range(N_CORES):
        ti, oj = divmod(c, OSPLIT)
        out[ti * TT : (ti + 1) * TT, oj * TO : (oj + 1) * TO] = res.results[c]["out"]
    out += bias  # exact final fp32 add (reference's last op)
    return out.reshape(B, S, O)


def kernel(x, weight, bias):
    return _run(x, weight, bias, trace=False)


def kernel_traced(x, weight, bias):
    _run(x, weight, bias, trace=True)
    return LAST_EXEC_NS


# revision 15
# speedup vs baseline: 1.0883x; 1.0020x over previous
# BitLinear (eval path) Trainium2 kernel: ternary weight quant + int8 activation
# quant + dense matmul, tensor-parallel over 8 NeuronCores.
#
# Math (per reference):
#   w_scale[o] = max(mean_k |W[o,k]|, EPS)
#   w_quant    = clip(round(W / w_scale), -1, 1)            (ternary)
#   x_scale[t] = max(max_k |x[t,k]| / 127, EPS)
#   x_quant    = round(x / x_scale)                          (int8 range)
#   out[t,o]   = (sum_k x_quant[t,k] * w_quant[o,k]) * x_scale[t] * w_scale[o] + bias[o]
#
# The integer sum is exact on the PE: w_quant and x_quant (|v| <= 127) are exact
# in bf16, partials are exact in the fp32 PSUM accumulator (|sum| <= 127*4096 < 2^24).
#
# Sharding: 2 token groups x 4 out-feature groups = 8 cores.  Both x and W are
# passed in natural row-major layout ([t,k] / [o,k]); quantization runs with the
# reduction dim on the free axis so scales are per-partition (cheap TS/ACT ops),
# then the quantized bf16 tiles are transposed to K-major with the DMA xbar
# transpose engine for the matmul.  Bias is added on the host (exact final fp32
# add; it is the last op of the reference).
#
# Scheduling: PE-bound main loop (2048 back-to-back N=512 bf16 matmuls,
# ~216ns each).  Per main-loop iteration the issue order keeps every queue
# free of head-of-line blocking: [matmuls(ch), STT(ch), out(ch)] then
# in-DMA(ch+6) then quant-chain(ch+4) (transpose dispatched last on sync).
# All input DMAs stay on the sync queue (DMA-in dispatched from the scalar
# queue alongside ACT compute proved flaky on hardware).
import numpy as np

import concourse.bacc as bacc
import concourse.bass as bass
import concourse.tile as tile
from concourse import mybir
from concourse.bass_utils import run_bass_kernel_spmd
from concourse.masks import make_identity

F32 = mybir.dt.float32
BF16 = mybir.dt.bfloat16

EPS = 1e-5
MAGIC = 12582912.0  # 1.5 * 2^23: (x + MAGIC) - MAGIC == rint(x) for |x| < 2^22

# Full-problem shapes (hardcoded per contract).
B, S, I, O = 4, 2048, 4096, 4096
T_FULL = B * S  # 8192 tokens
TSPLIT, OSPLIT = 2, 4  # token groups x out-feature groups = 8 cores
N_CORES = TSPLIT * OSPLIT

A = mybir.AluOpType
AF = mybir.ActivationFunctionType


def build_nc(K=I, TO=O // OSPLIT, TT=T_FULL // TSPLIT):
    """Per-core program: x [TT, K] f32, w [TO, K] f32 -> out [TT, TO] f32."""
    KT = K // 128  # 32 k subtiles
    NOT = TO // 128  # 8 weight-row tiles
    NCH = TT // 128  # 32 token chunks
    NOC = TO // 512  # 2 psum column chunks per token chunk
    QPRE = 4  # quantization lookahead (chunks)
    LPRE = 5  # x-load lookahead (chunks)

    nc = bacc.Bacc("TRN2", target_bir_lowering=False, debug=False)
    x_d = nc.dram_tensor("x", [TT, K], F32, kind="ExternalInput").ap()
    w_d = nc.dram_tensor("w", [TO, K], F32, kind="ExternalInput").ap()
    out_d = nc.dram_tensor("out", [TT, TO], F32, kind="ExternalOutput").ap()

    with tile.TileContext(nc) as tc:
        with (
            tc.tile_pool(name="blk", bufs=4) as p_blk,  # f32 staging (x chunks / w tiles)
            tc.tile_pool(name="yb", bufs=2) as p_yb,  # bf16 quantized, pre-transpose
            tc.tile_pool(name="absw", bufs=1) as p_absw,
            tc.tile_pool(name="wq", bufs=1) as p_wq,
            tc.tile_pool(name="xq", bufs=3) as p_xq,  # K-major xq chunks
            tc.tile_pool(name="osb", bufs=2) as p_osb,
            tc.tile_pool(name="xs", bufs=QPRE + 2) as p_xs,
            tc.tile_pool(name="small", bufs=4) as p_small,
            tc.tile_pool(name="const", bufs=1) as p_const,
            tc.tile_pool(name="ps_mm", bufs=6, space="PSUM") as ps_mm,
            tc.tile_pool(name="ps_warm", bufs=1, space="PSUM") as ps_warm_pool,
            tc.tile_pool(name="ps_misc", bufs=1, space="PSUM") as ps_misc,
        ):
            ident = p_const.tile([128, 128], F32)
            make_identity(nc, ident[:])
            junk = p_const.tile([128, 128], BF16)
            nc.vector.memset(junk[:], 1.0)
            # Resident quantized weights, K-major: [k%128, ot, kt, o%128].
            wq = p_wq.tile([128, NOT, KT, 128], BF16)
            ws_bc = p_const.tile([128, TO], F32)  # row 0 filled, then broadcast

            # ---------------- W phase ----------------
            def w_load(ot):
                w_f32 = p_blk.tile([128, K], F32, tag="blk")
                nc.sync.dma_start(
                    out=w_f32[:], in_=w_d[ot * 128 : (ot + 1) * 128, :]
                )
                return w_f32

            def w_quant(ot, w_f32):
                absw = p_absw.tile([128, K], F32, tag="absw")
                ws_sum = p_small.tile([128, 1], F32, tag="wsum")
                nc.scalar.activation(
                    out=absw[:], in_=w_f32[:], func=AF.Abs, accum_out=ws_sum[:]
                )
                ws_col = p_small.tile([128, 1], F32, tag="wscol")
                nc.vector.tensor_scalar(
                    out=ws_col[:], in0=ws_sum[:], scalar1=1.0 / K, scalar2=EPS,
                    op0=A.mult, op1=A.max,
                )
                rws_col = p_small.tile([128, 1], F32, tag="rwscol")
                nc.vector.reciprocal(rws_col[:], ws_col[:])
                # y = rint(w * rws) + MAGIC  (fp32, in place)
                nc.vector.tensor_scalar(
                    out=w_f32[:], in0=w_f32[:], scalar1=rws_col[:], scalar2=MAGIC,
                    op0=A.mult, op1=A.add,
                )
                yb = p_yb.tile([128, K], BF16, tag="yb")
                if ot % 2 == 0:
                    # ACT path: yb = y - MAGIC -> bf16 (exact affine), clip on DVE
                    nc.scalar.activation(
                        out=yb[:], in_=w_f32[:], func=AF.Copy, bias=-MAGIC
                    )
                    nc.vector.tensor_scalar(
                        out=yb[:], in0=yb[:], scalar1=1.0, scalar2=-1.0,
                        op0=A.min, op1=A.max,
                    )
                else:
                    # DVE path: two fp32 tensor_scalar passes
                    nc.vector.tensor_scalar(
                        out=w_f32[:], in0=w_f32[:], scalar1=MAGIC, scalar2=1.0,
                        op0=A.subtract, op1=A.min,
                    )
                    nc.vector.tensor_scalar(
                        out=yb[:], in0=w_f32[:], scalar1=-1.0, scalar2=None,
                        op0=A.max,
                    )
                # transpose [o, k] -> [k%128, kt, o] into the resident block
                nc.sync.dma_start_transpose(wq[:, ot, :, :], yb[:])
                # ws column -> ws_bc row-0 segment (PE transpose via identity)
                ps_r = ps_misc.tile([1, 128], F32, tag="wsrow")
                nc.tensor.transpose(ps_r[:], ws_col[:], ident[:])
                nc.vector.tensor_copy(
                    ws_bc[0:1, ot * 128 : (ot + 1) * 128], ps_r[:]
                )

            # ---------- x-chunk load + quantize ----------
            def x_load(ch):
                x_f32 = p_blk.tile([128, K], F32, tag="blk")
                nc.sync.dma_start(out=x_f32[:], in_=x_d[ch * 128 : (ch + 1) * 128, :])
                return x_f32

            def x_quant(ch, x_f32):
                am = p_small.tile([128, 1], F32, tag="am")
                nc.vector.tensor_reduce(
                    out=am[:], in_=x_f32[:], axis=mybir.AxisListType.X,
                    op=A.max, apply_absolute_value=True,
                )
                xs_col = p_xs.tile([128, 1], F32, tag="xs")
                nc.vector.tensor_scalar(
                    out=xs_col[:], in0=am[:], scalar1=1.0 / 127.0, scalar2=EPS,
                    op0=A.mult, op1=A.max,
                )
                rxs_col = p_small.tile([128, 1], F32, tag="rxs")
                nc.vector.reciprocal(rxs_col[:], xs_col[:])
                # y = rint(x * rxs) + MAGIC  (fp32, in place)
                nc.vector.tensor_scalar(
                    out=x_f32[:], in0=x_f32[:], scalar1=rxs_col[:], scalar2=MAGIC,
                    op0=A.mult, op1=A.add,
                )
                # yb = y - MAGIC -> bf16 (exact: integers, |v| <= 127)
                yb = p_yb.tile([128, K], BF16, tag="yb")
                nc.scalar.activation(out=yb[:], in_=x_f32[:], func=AF.Copy, bias=-MAGIC)
                # transpose [t, k] -> [k%128, kt, t]
                xq_kt = p_xq.tile([128, KT, 128], BF16, tag="xq")
                nc.sync.dma_start_transpose(xq_kt[:], yb[:])
                return xq_kt, xs_col

            # ---------- main chunk: matmuls + epilogue ----------
            def x_mainloop(ch, xq_kt, xs_col):
                osb = p_osb.tile([128, TO], F32, tag="osb")
                for oc in range(NOC):
                    pm = ps_mm.tile([128, 512], F32, tag="mm")
                    for kt in range(KT):
                        nc.tensor.matmul(
                            pm[:],
                            xq_kt[:, kt, :],
                            wq[:, oc * 4 : (oc + 1) * 4, kt, :],
                            start=(kt == 0),
                            stop=(kt == KT - 1),
                        )
                    # out = (psum * xs[t]) * ws[o]
                    nc.vector.scalar_tensor_tensor(
                        out=osb[:, oc * 512 : (oc + 1) * 512],
                        in0=pm[:],
                        scalar=xs_col[:],
                        in1=ws_bc[:, oc * 512 : (oc + 1) * 512],
                        op0=A.mult,
                        op1=A.mult,
                    )
                nc.scalar.dma_start(
                    out=out_d[ch * 128 : (ch + 1) * 128, :], in_=osb[:]
                )

            # ---------------- schedule ----------------
            # Head loads AND quant chains interleaved on the 4-deep blk ring
            # (w0 w1 x0 x1 w2 w3 | q(w0..1) q(x0..1) | x2 x3 | q(w2..3)
            # q(x2..3) | w4..w7 | q(w4..7)): every load's slot is freed by a
            # quant op already issued, on a different queue, whose own inputs
            # precede the load -- no cycles, and the early x chunks land well
            # before the W phase drains.
            w_tiles = {}
            x_tiles = {}
            quants = []
            w_tiles[0], w_tiles[1] = w_load(0), w_load(1)
            x_tiles[0], x_tiles[1] = x_load(0), x_load(1)
            w_tiles[2], w_tiles[3] = w_load(2), w_load(3)
            w_quant(0, w_tiles.pop(0))
            w_quant(1, w_tiles.pop(1))
            quants.append(x_quant(0, x_tiles.pop(0)))
            quants.append(x_quant(1, x_tiles.pop(1)))
            x_tiles[2], x_tiles[3] = x_load(2), x_load(3)
            w_quant(2, w_tiles.pop(2))
            w_quant(3, w_tiles.pop(3))
            quants.append(x_quant(2, x_tiles.pop(2)))
            quants.append(x_quant(3, x_tiles.pop(3)))
            for ot in range(4, NOT):
                w_tiles[ot] = w_load(ot)
            for ot in range(4, NOT):
                w_quant(ot, w_tiles.pop(ot))
            nc.gpsimd.partition_broadcast(ws_bc[:], ws_bc[0:1, :])
            for ch in (4, 5):
                x_tiles[ch] = x_load(ch)
            # PE warmup: junk matmuls gated on the second-to-last wq transpose,
            # bridging the HAM cold-ramp right before the main stream starts.
            ps_warm = ps_warm_pool.tile([128, 128], F32, tag="warm")
            for r in range(24):
                nc.tensor.matmul(
                    ps_warm[:], junk[:], wq[:, NOT - 2, r % KT, :],
                    start=True, stop=True,
                )
            warm_sink = p_small.tile([128, 1], F32, tag="wsink")
            nc.vector.tensor_copy(warm_sink[:], ps_warm[:, 0:1])
            for ch in range(NCH):
                xq_kt, xs_col = quants.pop(0)
                x_mainloop(ch, xq_kt, xs_col)
                if ch + LPRE + 1 < NCH:
                    x_tiles[ch + LPRE + 1] = x_load(ch + LPRE + 1)
                if ch + QPRE < NCH:
                    quants.append(x_quant(ch + QPRE, x_tiles.pop(ch + QPRE)))
    nc.compile()
    return nc


_NC_CACHE = {}
TRACE = False
LAST_EXEC_NS = None


def _get_nc():
    key = "full"
    if key not in _NC_CACHE:
        _NC_CACHE[key] = build_nc()
    return _NC_CACHE[key]


def _run(x, weight, bias, trace=False):
    global LAST_EXEC_NS
    x = np.asarray(x, dtype=np.float32)
    weight = np.asarray(weight, dtype=np.float32)
    bias = np.asarray(bias, dtype=np.float32)

    x2 = x.reshape(T_FULL, I)
    TT = T_FULL // TSPLIT
    TO = O // OSPLIT
    in_maps = []
    for c in range(N_CORES):
        ti, oj = divmod(c, OSPLIT)
        in_maps.append(
            {
                "x": np.ascontiguousarray(x2[ti * TT : (ti + 1) * TT]),
                "w": np.ascontiguousarray(weight[oj * TO : (oj + 1) * TO]),
            }
        )

    nc = _get_nc()
    res = run_bass_kernel_spmd(
        nc, in_maps, core_ids=list(range(N_CORES)), trace=trace
    )
    LAST_EXEC_NS = res.exec_time_ns

    out = np.empty((T_FULL, O), dtype=np.float32)
    for c in range(N_CORES):
        ti, oj = divmod(c, OSPLIT)
        out[ti * TT : (ti + 1) * TT, oj * TO : (oj + 1) * TO] = res.results[c]["out"]
    out += bias  # exact final fp32 add (reference's last op)
    return out.reshape(B, S, O)


def kernel(x, weight, bias):
    return _run(x, weight, bias, trace=False)


def kernel_traced(x, weight, bias):
    _run(x, weight, bias, trace=True)
    return LAST_EXEC_NS
